# revision 12
# baseline (speedup 1.0000x reference)
"""Trainium2 Bass kernel for nn_AddTaskModel (adaptive-threshold spiking RNN).

Data-parallel over 8 NeuronCores: batch 2048 -> 256/core -> 2 staggered
blocks of 128.  Layout [H=128 partitions, batch on free dim].  The scan
runs fully on-device in fp32 (matmuls fp32; the input projection and the
spike matmul use exact/bf16-split fast paths); the final tiny readout
(mem @ Wlin.T + blin vs y) is done on host from the gathered final
membrane state.

Self-contained: only needs numpy + the concourse (bass) runtime at
/opt/trn_rl_repo and 8 axon-tunneled TRN2 cores.
"""
import sys
if "/opt/trn_rl_repo" not in sys.path:
    sys.path.insert(0, "/opt/trn_rl_repo")

import numpy as np
import ml_dtypes
import concourse.bass as bass
import concourse.mybir as mybir
from concourse import tile

F32 = mybir.dt.float32
F32R = mybir.dt.float32r
BF16 = mybir.dt.bfloat16
ALU = mybir.AluOpType
AF = mybir.ActivationFunctionType

B_J0 = 0.01
BETA = 1.8

H = 128            # hidden
S = 1024           # timesteps
B_FULL = 2048      # full batch
N_CORES = 8
BSH = B_FULL // N_CORES   # 256 per core
FB = 128                  # batch per block
NBLK = 2
X_CHUNK = 16              # steps per dense_x DMA chunk


def _split_multiwaits(nc, max_waits=1):
    """Walrus codegen allows at most one sync wait per instruction; hoist
    extras into standalone EventSemaphore waits on the same engine queue."""
    for f in nc.m.functions:
        for blk in f.blocks:
            newlist = []
            for ins in blk.instructions:
                si = ins.sync_info
                if si is not None and si.on_wait and len(si.on_wait) > max_waits:
                    waits = list(si.on_wait)
                    for k, w in enumerate(waits[:-max_waits]):
                        ev = mybir.InstEventSemaphore(
                            name=f"{ins.name}_xw{k}", ins=[], outs=[])
                        ev.engine = ins.engine
                        ev.sync_info = mybir.SyncInfo(on_wait=[w], on_update=[])
                        newlist.append(ev)
                    ins.sync_info = mybir.SyncInfo(
                        on_wait=waits[-max_waits:],
                        on_update=list(si.on_update or []))
                newlist.append(ins)
            blk.instructions = newlist


def _build_nc(x_chunk=X_CHUNK):
    nc = bass.Bass()
    xts = [nc.declare_dram_parameter(f"xt{b}", [10, S * FB], BF16, isOutput=False)
           for b in range(NBLK)]
    wx2d = nc.declare_dram_parameter("wx2", [10, H], BF16, isOutput=False)
    NP32 = 4 * H + 2 + NBLK * 2 * FB
    wpack = nc.declare_dram_parameter("wpack", [H, NP32], F32, isOutput=False)
    wpackr = nc.declare_dram_parameter("wpackr", [H, 4 * H], F32R, isOutput=False)
    NPBF = H + NBLK * FB
    bpack = nc.declare_dram_parameter("bpack", [H, NPBF], BF16, isOutput=False)
    mem_out = nc.declare_dram_parameter("mem_out", [H, BSH], F32, isOutput=True)

    with tile.TileContext(nc) as tc:
        with (
            tc.tile_pool(name="const", bufs=1) as constp,
            tc.tile_pool(name="xin", bufs=3) as xinp,
            tc.tile_pool(name="spk", bufs=3) as spkp,
            tc.tile_pool(name="state", bufs=3) as statep,
            tc.tile_pool(name="stater", bufs=3) as staterp,
            tc.tile_pool(name="dense", bufs=3) as densep,
            tc.tile_pool(name="tau", bufs=3) as taup,
            tc.tile_pool(name="pp", bufs=3) as ppp,
            tc.tile_pool(name="tmp", bufs=3) as tmpp,
            tc.tile_pool(name="ps", bufs=1, space="PSUM") as psp,
            tc.tile_pool(name="psx", bufs=2, space="PSUM") as psxp,
        ):
            wsb = constp.tile([H, NP32], F32)
            nc.sync.dma_start(wsb[:], wpack[:])
            wrb = constp.tile([H, 4 * H], F32R)
            nc.sync.dma_start(wrb[:], wpackr[:])
            w_ma = wrb[:, 0 * H:1 * H]
            w_mb = wrb[:, 1 * H:2 * H]
            w_aa = wrb[:, 2 * H:3 * H]
            w_ab = wrb[:, 3 * H:4 * H]
            btauM = wsb[:, 4 * H + 0:4 * H + 1]
            btauA = wsb[:, 4 * H + 1:4 * H + 2]
            c0 = 4 * H + 2
            st = [wsb[:, c0 + b * 2 * FB: c0 + (b + 1) * 2 * FB] for b in range(NBLK)]
            str_ = []
            for b in range(NBLK):
                r0 = staterp.tile([H, 2 * FB], F32R, tag=f"str{b}",
                                  name=f"str{b}_init")
                nc.vector.tensor_scalar(r0[:], st[b], 0.0, None, ALU.add)
                str_.append(r0)

            bsb = constp.tile([H, NPBF], BF16)
            nc.sync.dma_start(bsb[:], bpack[:])
            w_xb = bsb[:, 0:H]
            spk = [bsb[:, H + b * FB:H + (b + 1) * FB] for b in range(NBLK)]
            wx2 = constp.tile([10, H], BF16)
            nc.sync.dma_start(wx2[:], wx2d[:])

            psM = [psp.tile([H, FB], F32, tag=f"psM{b}", name=f"psM{b}") for b in range(NBLK)]
            psA = [psp.tile([H, FB], F32, tag=f"psA{b}", name=f"psA{b}") for b in range(NBLK)]

            xbh = [None, None]
            for t in range(S):
                if t % x_chunk == 0:
                    n_steps = min(x_chunk, S - t)
                    for b in range(NBLK):
                        xbh[b] = xinp.tile([10, x_chunk * FB], BF16, tag=f"xh{b}", name=f"xh{b}_{t}")
                        nc.sync.dma_start(xbh[b][:, 0:n_steps * FB],
                                          xts[b][:, t * FB:(t + n_steps) * FB])
                for b in range(NBLK):
                    bs = st[b][:, 0:FB]
                    mem = st[b][:, FB:2 * FB]
                    bsr = str_[b][:, 0:FB]
                    memr = str_[b][:, FB:2 * FB]
                    spk_b = spk[b]
                    co = (t % x_chunk) * FB
                    xt = xbh[b][:, co:co + FB]

                    # psX = (W@x + b1x) via K=10 packed trick + Wxb@spk
                    psX = psxp.tile([H, FB], F32, tag=f"psX{b}", name=f"psX{b}_{t}")
                    nc.tensor.matmul(psX[:], wx2[:], xt, start=True, stop=False)
                    nc.tensor.matmul(psX[:], w_xb, spk_b[:], start=False, stop=True)

                    dense = densep.tile([H, FB], F32, tag=f"dense{b}")
                    nc.scalar.activation(dense[:], psX[:], AF.Copy)
                    denser = densep.tile([H, FB], F32R, tag=f"denser{b}",
                                         name=f"denser{b}_{t}")
                    nc.scalar.activation(denser[:], psX[:], AF.Copy)

                    nc.tensor.matmul(psM[b][:], w_mb, memr, start=True, stop=False)
                    nc.tensor.matmul(psA[b][:], w_ab, bsr, start=True, stop=False)
                    nc.tensor.matmul(psM[b][:], w_ma, denser[:], start=False, stop=True)
                    nc.tensor.matmul(psA[b][:], w_aa, denser[:], start=False, stop=True)

                    tauMA = taup.tile([H, 2 * FB], F32, tag=f"tau{b}")  # [tauA | tauM]
                    nc.scalar.activation(tauMA[:, FB:2 * FB], psM[b][:], AF.Sigmoid, bias=btauM)
                    nc.scalar.activation(tauMA[:, 0:FB], psA[b][:], AF.Sigmoid, bias=btauA)

                    dd = ppp.tile([H, 2 * FB], F32, tag=f"dd{b}")  # [d1s | d2]
                    nc.vector.scalar_tensor_tensor(
                        dd[:, 0:FB], spk_b[:], -BETA, bs, ALU.mult, ALU.add)
                    nc.vector.scalar_tensor_tensor(
                        dd[:, FB:2 * FB], dense[:], -1.0, mem, ALU.mult, ALU.add)
                    pp = ppp.tile([H, 2 * FB], F32, tag=f"pp{b}")  # [p1s | p2]
                    nc.vector.scalar_tensor_tensor(pp[:], tauMA[:], 1.0, dd[:], ALU.mult, ALU.mult)
                    p1s = pp[:, 0:FB]
                    p2 = pp[:, FB:2 * FB]

                    st_n = statep.tile([H, 2 * FB], F32, tag=f"state{b}")
                    bs_n = st_n[:, 0:FB]
                    mem_n = st_n[:, FB:2 * FB]
                    s1 = tmpp.tile([H, FB], F32, tag=f"s1{b}")
                    nc.vector.scalar_tensor_tensor(s1[:], dense[:], 1.0, p2, ALU.mult, ALU.add)
                    # t6 = Bth*spk = (p1s + BETA + B_J0) * spk   (spk in {0,1})
                    t6 = tmpp.tile([H, FB], F32, tag=f"t6{b}")
                    nc.vector.scalar_tensor_tensor(
                        t6[:], p1s, BETA + B_J0, spk_b[:], ALU.add, ALU.mult)
                    nc.vector.scalar_tensor_tensor(
                        bs_n, spk_b[:], BETA, p1s, ALU.mult, ALU.add)
                    # flush tiny bb values away from denormal range (DVE slowdown);
                    # Bth = 0.01 + bs so a 1e-10 floor is numerically invisible
                    nc.vector.tensor_scalar(bs_n, bs_n, 1e-10, None, ALU.max)
                    nc.vector.scalar_tensor_tensor(
                        mem_n, t6[:], -1.0, s1[:], ALU.mult, ALU.add)
                    spk_n = spkp.tile([H, FB], BF16, tag=f"spk{b}")
                    nc.vector.scalar_tensor_tensor(
                        spk_n[:], bs_n, B_J0, mem_n, ALU.add, ALU.is_lt)

                    str_n = staterp.tile([H, 2 * FB], F32R, tag=f"str{b}",
                                         name=f"strn{b}_{t}")
                    nc.vector.tensor_scalar(str_n[:], st_n[:], 0.0, None, ALU.add)

                    spk[b] = spk_n
                    st[b] = st_n
                    str_[b] = str_n

            for b in range(NBLK):
                nc.sync.dma_start(mem_out[:, b * FB:(b + 1) * FB], st[b][:, FB:2 * FB])

    _split_multiwaits(nc)
    return nc


def _bf16_split(a):
    hi = a.astype(ml_dtypes.bfloat16)
    lo = (a - hi.astype(np.float32)).astype(ml_dtypes.bfloat16)
    return hi, lo


def _prep_inputs_per_core(inputs):
    x = np.asarray(inputs["x"], np.float32)          # [S, B, 2]
    W1x = np.asarray(inputs["W1x"], np.float32)
    b1x = np.asarray(inputs["b1x"], np.float32)
    WtauM = np.asarray(inputs["WtauM"], np.float32)
    WtauAdp = np.asarray(inputs["WtauAdp"], np.float32)
    biases = np.stack([np.asarray(inputs["btauM"], np.float32),
                       np.asarray(inputs["btauAdp"], np.float32)], axis=1)
    wpack_l = [WtauM[:, :128].T, WtauM[:, 128:].T,
               WtauAdp[:, :128].T, (WtauAdp[:, 128:] / BETA).T, biases]
    wxb = np.ascontiguousarray(W1x[:, 2:].T).astype(ml_dtypes.bfloat16)
    h0_mem = np.asarray(inputs["h0_mem"], np.float32)
    h0_spk = np.asarray(inputs["h0_spk"], np.float32)
    h0_b = np.asarray(inputs["h0_b"], np.float32)

    # K=10: [Wh;Wh;Wl;Wl;b1h;b1l] x [xh;xl;xh;xl;1;1] = W@x + b1x exactly
    w2h, w2l = _bf16_split(np.ascontiguousarray(W1x[:, :2].T, np.float32))
    b1h, b1l = _bf16_split(b1x)
    wx2 = np.ascontiguousarray(np.concatenate(
        [w2h, w2h, w2l, w2l, b1h[None, :], b1l[None, :]], axis=0))
    xT = x.transpose(2, 0, 1)                                # [2, S, B]
    xh, xl = _bf16_split(xT)
    onesrow = np.ones((2, S * FB), dtype=ml_dtypes.bfloat16)

    in_maps = []
    for c in range(N_CORES):
        m = {"wx2": wx2}
        p32 = list(wpack_l)
        pbf = [wxb]
        for b in range(NBLK):
            sl = slice(c * BSH + b * FB, c * BSH + (b + 1) * FB)
            xpk = np.concatenate([xh[:, :, sl], xl[:, :, sl],
                                  xh[:, :, sl], xl[:, :, sl]], axis=0)
            m[f"xt{b}"] = np.ascontiguousarray(
                np.concatenate([xpk.reshape(8, S * FB), onesrow], axis=0))
            p32.append(np.concatenate([(BETA * h0_b[sl]).T, h0_mem[sl].T], axis=1))
            pbf.append(h0_spk[sl].T.astype(ml_dtypes.bfloat16))
        m["wpack"] = np.ascontiguousarray(np.concatenate(p32, axis=1).astype(np.float32))
        m["wpackr"] = np.ascontiguousarray(m["wpack"][:, :4 * H])
        m["bpack"] = np.ascontiguousarray(
            np.concatenate([np.asarray(a, dtype=ml_dtypes.bfloat16) for a in pbf], axis=1))
        in_maps.append(m)
    return in_maps


_NC_CACHE = {}


def _get_nc():
    if "nc" not in _NC_CACHE:
        _NC_CACHE["nc"] = _build_nc()
    return _NC_CACHE["nc"]


def _run(inputs, trace=False):
    from concourse.bass_utils import run_bass_kernel_spmd
    nc = _get_nc()
    in_maps = _prep_inputs_per_core(inputs)
    res = run_bass_kernel_spmd(nc, in_maps, core_ids=list(range(N_CORES)),
                               trace=trace)
    return res


def _finish_host(results, inputs):
    Wlin = np.asarray(inputs["Wlin"], np.float32)
    blin = np.asarray(inputs["blin"], np.float32)
    y = np.asarray(inputs["y"], np.float32)
    mem = np.concatenate([r["mem_out"].T for r in results], axis=0)  # [B, H]
    out = (mem @ Wlin.T + blin)[:, 0]
    return np.float32(np.mean((out.astype(np.float32) - y) ** 2, dtype=np.float32))


def kernel(x, y, h0_mem, h0_spk, h0_b, W1x, b1x, WtauM, btauM, WtauAdp,
           btauAdp, Wlin, blin):
    """Full (unsharded) inputs -> full scalar loss, computed on 8 TRN2 cores."""
    inputs = dict(x=x, y=y, h0_mem=h0_mem, h0_spk=h0_spk, h0_b=h0_b,
                  W1x=W1x, b1x=b1x, WtauM=WtauM, btauM=btauM,
                  WtauAdp=WtauAdp, btauAdp=btauAdp, Wlin=Wlin, blin=blin)
    res = _run(inputs, trace=False)
    return _finish_host(res.results, inputs)


def kernel_profiled(**inputs):
    """Like kernel(), but also returns neuron-profile exec time in ns."""
    res = _run(inputs, trace=True)
    return _finish_host(res.results, inputs), res.exec_time_ns



# revision 13
# speedup vs baseline: 1.0306x; 1.0306x over previous
"""Trainium2 Bass kernel for nn_AddTaskModel (adaptive-threshold spiking RNN).

Data-parallel over 8 NeuronCores: batch 2048 -> 256/core -> 2 staggered
blocks of 128.  Layout [H=128 partitions, batch on free dim].  The scan
runs fully on-device in fp32 (matmuls fp32; the input projection and the
spike matmul use exact/bf16-split fast paths); the final tiny readout
(mem @ Wlin.T + blin vs y) is done on host from the gathered final
membrane state.

Self-contained: only needs numpy + the concourse (bass) runtime at
/opt/trn_rl_repo and 8 axon-tunneled TRN2 cores.
"""
import sys
if "/opt/trn_rl_repo" not in sys.path:
    sys.path.insert(0, "/opt/trn_rl_repo")

import numpy as np
import ml_dtypes
import concourse.bass as bass
import concourse.mybir as mybir
from concourse import tile

F32 = mybir.dt.float32
F32R = mybir.dt.float32r
BF16 = mybir.dt.bfloat16
ALU = mybir.AluOpType
AF = mybir.ActivationFunctionType

B_J0 = 0.01
BETA = 1.8

H = 128            # hidden
S = 1024           # timesteps
B_FULL = 2048      # full batch
N_CORES = 8
BSH = B_FULL // N_CORES   # 256 per core
FB = 128                  # batch per block
NBLK = 2
X_CHUNK = 32              # steps per x DMA chunk


def _split_multiwaits(nc, max_waits=1):
    """Walrus codegen allows at most one sync wait per instruction; hoist
    extras into standalone EventSemaphore waits on the same engine queue."""
    for f in nc.m.functions:
        for blk in f.blocks:
            newlist = []
            for ins in blk.instructions:
                si = ins.sync_info
                if si is not None and si.on_wait and len(si.on_wait) > max_waits:
                    waits = list(si.on_wait)
                    for k, w in enumerate(waits[:-max_waits]):
                        ev = mybir.InstEventSemaphore(
                            name=f"{ins.name}_xw{k}", ins=[], outs=[])
                        ev.engine = ins.engine
                        ev.sync_info = mybir.SyncInfo(on_wait=[w], on_update=[])
                        newlist.append(ev)
                    ins.sync_info = mybir.SyncInfo(
                        on_wait=waits[-max_waits:],
                        on_update=list(si.on_update or []))
                newlist.append(ins)
            blk.instructions = newlist


def _build_nc(x_chunk=X_CHUNK):
    nc = bass.Bass()
    xts = [nc.declare_dram_parameter(f"xt{b}", [10, S * FB], BF16, isOutput=False)
           for b in range(NBLK)]
    wx2d = nc.declare_dram_parameter("wx2", [10, H], BF16, isOutput=False)
    NP32 = 4 * H + 2 + NBLK * 2 * FB
    wpack = nc.declare_dram_parameter("wpack", [H, NP32], F32, isOutput=False)
    wpackr = nc.declare_dram_parameter("wpackr", [H, 4 * H], F32R, isOutput=False)
    NPBF = H + NBLK * FB
    bpack = nc.declare_dram_parameter("bpack", [H, NPBF], BF16, isOutput=False)
    mem_out = nc.declare_dram_parameter("mem_out", [H, BSH], F32, isOutput=True)

    with tile.TileContext(nc) as tc:
        with (
            tc.tile_pool(name="const", bufs=1) as constp,
            tc.tile_pool(name="xin", bufs=3) as xinp,
            tc.tile_pool(name="spk", bufs=3) as spkp,
            tc.tile_pool(name="state", bufs=3) as statep,
            tc.tile_pool(name="stater", bufs=3) as staterp,
            tc.tile_pool(name="dense", bufs=3) as densep,
            tc.tile_pool(name="tau", bufs=3) as taup,
            tc.tile_pool(name="pp", bufs=3) as ppp,
            tc.tile_pool(name="tmp", bufs=3) as tmpp,
            tc.tile_pool(name="ps", bufs=1, space="PSUM") as psp,
            tc.tile_pool(name="psx", bufs=2, space="PSUM") as psxp,
        ):
            wsb = constp.tile([H, NP32], F32)
            nc.sync.dma_start(wsb[:], wpack[:])
            wrb = constp.tile([H, 4 * H], F32R)
            nc.sync.dma_start(wrb[:], wpackr[:])
            w_ma = wrb[:, 0 * H:1 * H]
            w_mb = wrb[:, 1 * H:2 * H]
            w_aa = wrb[:, 2 * H:3 * H]
            w_ab = wrb[:, 3 * H:4 * H]
            btauM = wsb[:, 4 * H + 0:4 * H + 1]
            btauA = wsb[:, 4 * H + 1:4 * H + 2]
            c0 = 4 * H + 2
            st = [wsb[:, c0 + b * 2 * FB: c0 + (b + 1) * 2 * FB] for b in range(NBLK)]
            str_ = []
            for b in range(NBLK):
                r0 = staterp.tile([H, 2 * FB], F32R, tag=f"str{b}",
                                  name=f"str{b}_init")
                nc.vector.tensor_scalar(r0[:], st[b], 0.0, None, ALU.add)
                str_.append(r0)

            bsb = constp.tile([H, NPBF], BF16)
            nc.sync.dma_start(bsb[:], bpack[:])
            w_xb = bsb[:, 0:H]
            spk = [bsb[:, H + b * FB:H + (b + 1) * FB] for b in range(NBLK)]
            wx2 = constp.tile([10, H], BF16)
            nc.sync.dma_start(wx2[:], wx2d[:])

            psM = [psp.tile([H, FB], F32, tag=f"psM{b}", name=f"psM{b}") for b in range(NBLK)]
            psA = [psp.tile([H, FB], F32, tag=f"psA{b}", name=f"psA{b}") for b in range(NBLK)]

            xbh = [None, None]
            for t in range(S):
                if t % x_chunk == 0:
                    n_steps = min(x_chunk, S - t)
                    for b in range(NBLK):
                        xbh[b] = xinp.tile([10, x_chunk * FB], BF16, tag=f"xh{b}", name=f"xh{b}_{t}")
                        nc.sync.dma_start(xbh[b][:, 0:n_steps * FB],
                                          xts[b][:, t * FB:(t + n_steps) * FB])
                for b in range(NBLK):
                    bs = st[b][:, 0:FB]
                    mem = st[b][:, FB:2 * FB]
                    bsr = str_[b][:, 0:FB]
                    memr = str_[b][:, FB:2 * FB]
                    spk_b = spk[b]
                    co = (t % x_chunk) * FB
                    xt = xbh[b][:, co:co + FB]

                    # psX = (W@x + b1x) via K=10 packed trick + Wxb@spk
                    psX = psxp.tile([H, FB], F32, tag=f"psX{b}", name=f"psX{b}_{t}")
                    nc.tensor.matmul(psX[:], wx2[:], xt, start=True, stop=False)
                    nc.tensor.matmul(psX[:], w_xb, spk_b[:], start=False, stop=True)

                    dense = densep.tile([H, FB], F32, tag=f"dense{b}")
                    nc.scalar.activation(dense[:], psX[:], AF.Copy)
                    denser = densep.tile([H, FB], F32R, tag=f"denser{b}",
                                         name=f"denser{b}_{t}")
                    nc.scalar.activation(denser[:], psX[:], AF.Copy)

                    nc.tensor.matmul(psM[b][:], w_mb, memr, start=True, stop=False)
                    nc.tensor.matmul(psA[b][:], w_ab, bsr, start=True, stop=False)
                    nc.tensor.matmul(psM[b][:], w_ma, denser[:], start=False, stop=True)
                    nc.tensor.matmul(psA[b][:], w_aa, denser[:], start=False, stop=True)

                    tauMA = taup.tile([H, 2 * FB], F32, tag=f"tau{b}")  # [tauA | tauM]
                    nc.scalar.activation(tauMA[:, FB:2 * FB], psM[b][:], AF.Sigmoid, bias=btauM)
                    nc.scalar.activation(tauMA[:, 0:FB], psA[b][:], AF.Sigmoid, bias=btauA)

                    dd = ppp.tile([H, 2 * FB], F32, tag=f"dd{b}")  # [d1s | d2]
                    nc.vector.scalar_tensor_tensor(
                        dd[:, 0:FB], spk_b[:], -BETA, bs, ALU.mult, ALU.add)
                    nc.vector.scalar_tensor_tensor(
                        dd[:, FB:2 * FB], dense[:], -1.0, mem, ALU.mult, ALU.add)
                    pp = ppp.tile([H, 2 * FB], F32, tag=f"pp{b}")  # [p1s | p2]
                    nc.vector.scalar_tensor_tensor(pp[:], tauMA[:], 1.0, dd[:], ALU.mult, ALU.mult)
                    p1s = pp[:, 0:FB]
                    p2 = pp[:, FB:2 * FB]

                    st_n = statep.tile([H, 2 * FB], F32, tag=f"state{b}")
                    bs_n = st_n[:, 0:FB]
                    mem_n = st_n[:, FB:2 * FB]
                    s1 = tmpp.tile([H, FB], F32, tag=f"s1{b}")
                    nc.vector.scalar_tensor_tensor(s1[:], dense[:], 1.0, p2, ALU.mult, ALU.add)
                    # t6 = Bth*spk = (p1s + BETA + B_J0) * spk   (spk in {0,1})
                    t6 = tmpp.tile([H, FB], F32, tag=f"t6{b}")
                    nc.vector.scalar_tensor_tensor(
                        t6[:], p1s, BETA + B_J0, spk_b[:], ALU.add, ALU.mult)
                    nc.vector.scalar_tensor_tensor(
                        bs_n, spk_b[:], BETA, p1s, ALU.mult, ALU.add)
                    # flush tiny bb values away from denormal range (DVE slowdown);
                    # Bth = 0.01 + bs so a 1e-10 floor is numerically invisible
                    nc.vector.tensor_scalar(bs_n, bs_n, 1e-10, None, ALU.max)
                    nc.vector.scalar_tensor_tensor(
                        mem_n, t6[:], -1.0, s1[:], ALU.mult, ALU.add)
                    spk_n = spkp.tile([H, FB], BF16, tag=f"spk{b}")
                    nc.vector.scalar_tensor_tensor(
                        spk_n[:], bs_n, B_J0, mem_n, ALU.add, ALU.is_lt)

                    str_n = staterp.tile([H, 2 * FB], F32R, tag=f"str{b}",
                                         name=f"strn{b}_{t}")
                    nc.scalar.activation(str_n[:], st_n[:], AF.Copy)

                    spk[b] = spk_n
                    st[b] = st_n
                    str_[b] = str_n

            for b in range(NBLK):
                nc.sync.dma_start(mem_out[:, b * FB:(b + 1) * FB], st[b][:, FB:2 * FB])

    _split_multiwaits(nc)
    return nc


def _bf16_split(a):
    hi = a.astype(ml_dtypes.bfloat16)
    lo = (a - hi.astype(np.float32)).astype(ml_dtypes.bfloat16)
    return hi, lo


def _prep_inputs_per_core(inputs):
    x = np.asarray(inputs["x"], np.float32)          # [S, B, 2]
    W1x = np.asarray(inputs["W1x"], np.float32)
    b1x = np.asarray(inputs["b1x"], np.float32)
    WtauM = np.asarray(inputs["WtauM"], np.float32)
    WtauAdp = np.asarray(inputs["WtauAdp"], np.float32)
    biases = np.stack([np.asarray(inputs["btauM"], np.float32),
                       np.asarray(inputs["btauAdp"], np.float32)], axis=1)
    wpack_l = [WtauM[:, :128].T, WtauM[:, 128:].T,
               WtauAdp[:, :128].T, (WtauAdp[:, 128:] / BETA).T, biases]
    wxb = np.ascontiguousarray(W1x[:, 2:].T).astype(ml_dtypes.bfloat16)
    h0_mem = np.asarray(inputs["h0_mem"], np.float32)
    h0_spk = np.asarray(inputs["h0_spk"], np.float32)
    h0_b = np.asarray(inputs["h0_b"], np.float32)

    # K=10: [Wh;Wh;Wl;Wl;b1h;b1l] x [xh;xl;xh;xl;1;1] = W@x + b1x exactly
    w2h, w2l = _bf16_split(np.ascontiguousarray(W1x[:, :2].T, np.float32))
    b1h, b1l = _bf16_split(b1x)
    wx2 = np.ascontiguousarray(np.concatenate(
        [w2h, w2h, w2l, w2l, b1h[None, :], b1l[None, :]], axis=0))
    xT = x.transpose(2, 0, 1)                                # [2, S, B]
    xh, xl = _bf16_split(xT)
    onesrow = np.ones((2, S * FB), dtype=ml_dtypes.bfloat16)

    in_maps = []
    for c in range(N_CORES):
        m = {"wx2": wx2}
        p32 = list(wpack_l)
        pbf = [wxb]
        for b in range(NBLK):
            sl = slice(c * BSH + b * FB, c * BSH + (b + 1) * FB)
            xpk = np.concatenate([xh[:, :, sl], xl[:, :, sl],
                                  xh[:, :, sl], xl[:, :, sl]], axis=0)
            m[f"xt{b}"] = np.ascontiguousarray(
                np.concatenate([xpk.reshape(8, S * FB), onesrow], axis=0))
            p32.append(np.concatenate([(BETA * h0_b[sl]).T, h0_mem[sl].T], axis=1))
            pbf.append(h0_spk[sl].T.astype(ml_dtypes.bfloat16))
        m["wpack"] = np.ascontiguousarray(np.concatenate(p32, axis=1).astype(np.float32))
        m["wpackr"] = np.ascontiguousarray(m["wpack"][:, :4 * H])
        m["bpack"] = np.ascontiguousarray(
            np.concatenate([np.asarray(a, dtype=ml_dtypes.bfloat16) for a in pbf], axis=1))
        in_maps.append(m)
    return in_maps


_NC_CACHE = {}


def _get_nc():
    if "nc" not in _NC_CACHE:
        _NC_CACHE["nc"] = _build_nc()
    return _NC_CACHE["nc"]


def _run(inputs, trace=False):
    from concourse.bass_utils import run_bass_kernel_spmd
    nc = _get_nc()
    in_maps = _prep_inputs_per_core(inputs)
    res = run_bass_kernel_spmd(nc, in_maps, core_ids=list(range(N_CORES)),
                               trace=trace)
    return res


def _finish_host(results, inputs):
    Wlin = np.asarray(inputs["Wlin"], np.float32)
    blin = np.asarray(inputs["blin"], np.float32)
    y = np.asarray(inputs["y"], np.float32)
    mem = np.concatenate([r["mem_out"].T for r in results], axis=0)  # [B, H]
    out = (mem @ Wlin.T + blin)[:, 0]
    return np.float32(np.mean((out.astype(np.float32) - y) ** 2, dtype=np.float32))


def kernel(x, y, h0_mem, h0_spk, h0_b, W1x, b1x, WtauM, btauM, WtauAdp,
           btauAdp, Wlin, blin):
    """Full (unsharded) inputs -> full scalar loss, computed on 8 TRN2 cores."""
    inputs = dict(x=x, y=y, h0_mem=h0_mem, h0_spk=h0_spk, h0_b=h0_b,
                  W1x=W1x, b1x=b1x, WtauM=WtauM, btauM=btauM,
                  WtauAdp=WtauAdp, btauAdp=btauAdp, Wlin=Wlin, blin=blin)
    res = _run(inputs, trace=False)
    return _finish_host(res.results, inputs)


def kernel_profiled(**inputs):
    """Like kernel(), but also returns neuron-profile exec time in ns."""
    res = _run(inputs, trace=True)
    return _finish_host(res.results, inputs), res.exec_time_ns



# revision 14
# speedup vs baseline: 1.0326x; 1.0020x over previous
"""Trainium2 Bass kernel for nn_AddTaskModel (adaptive-threshold spiking RNN).

Data-parallel over 8 NeuronCores: batch 2048 -> 256/core -> 2 staggered
blocks of 128.  Layout [H=128 partitions, batch on free dim].  The scan
runs fully on-device in fp32 (matmuls fp32; the input projection and the
spike matmul use exact/bf16-split fast paths); the final tiny readout
(mem @ Wlin.T + blin vs y) is done on host from the gathered final
membrane state.

Self-contained: only needs numpy + the concourse (bass) runtime at
/opt/trn_rl_repo and 8 axon-tunneled TRN2 cores.
"""
import sys
if "/opt/trn_rl_repo" not in sys.path:
    sys.path.insert(0, "/opt/trn_rl_repo")

import numpy as np
import ml_dtypes
import concourse.bass as bass
import concourse.mybir as mybir
from concourse import tile

F32 = mybir.dt.float32
F32R = mybir.dt.float32r
BF16 = mybir.dt.bfloat16
ALU = mybir.AluOpType
AF = mybir.ActivationFunctionType

B_J0 = 0.01
BETA = 1.8

H = 128            # hidden
S = 1024           # timesteps
B_FULL = 2048      # full batch
N_CORES = 8
BSH = B_FULL // N_CORES   # 256 per core
FB = 128                  # batch per block
NBLK = 2
X_CHUNK = 32              # steps per x DMA chunk


def _split_multiwaits(nc, max_waits=1):
    """Walrus codegen allows at most one sync wait per instruction; hoist
    extras into standalone EventSemaphore waits on the same engine queue."""
    for f in nc.m.functions:
        for blk in f.blocks:
            newlist = []
            for ins in blk.instructions:
                si = ins.sync_info
                if si is not None and si.on_wait and len(si.on_wait) > max_waits:
                    waits = list(si.on_wait)
                    for k, w in enumerate(waits[:-max_waits]):
                        ev = mybir.InstEventSemaphore(
                            name=f"{ins.name}_xw{k}", ins=[], outs=[])
                        ev.engine = ins.engine
                        ev.sync_info = mybir.SyncInfo(on_wait=[w], on_update=[])
                        newlist.append(ev)
                    ins.sync_info = mybir.SyncInfo(
                        on_wait=waits[-max_waits:],
                        on_update=list(si.on_update or []))
                newlist.append(ins)
            blk.instructions = newlist


def _build_nc(x_chunk=X_CHUNK):
    nc = bass.Bass()
    xts = [nc.declare_dram_parameter(f"xt{b}", [10, S * FB], BF16, isOutput=False)
           for b in range(NBLK)]
    wx2d = nc.declare_dram_parameter("wx2", [10, H], BF16, isOutput=False)
    NP32 = 4 * H + 2 + NBLK * 2 * FB
    wpack = nc.declare_dram_parameter("wpack", [H, NP32], F32, isOutput=False)
    wpackr = nc.declare_dram_parameter("wpackr", [H, 4 * H], F32R, isOutput=False)
    NPBF = H + NBLK * FB
    bpack = nc.declare_dram_parameter("bpack", [H, NPBF], BF16, isOutput=False)
    mem_out = nc.declare_dram_parameter("mem_out", [H, BSH], F32, isOutput=True)

    with tile.TileContext(nc) as tc:
        with (
            tc.tile_pool(name="const", bufs=1) as constp,
            tc.tile_pool(name="xin", bufs=3) as xinp,
            tc.tile_pool(name="spk", bufs=3) as spkp,
            tc.tile_pool(name="state", bufs=3) as statep,
            tc.tile_pool(name="stater", bufs=3) as staterp,
            tc.tile_pool(name="dense", bufs=3) as densep,
            tc.tile_pool(name="tau", bufs=3) as taup,
            tc.tile_pool(name="pp", bufs=3) as ppp,
            tc.tile_pool(name="tmp", bufs=3) as tmpp,
            tc.tile_pool(name="ps", bufs=1, space="PSUM") as psp,
            tc.tile_pool(name="psx", bufs=2, space="PSUM") as psxp,
        ):
            wsb = constp.tile([H, NP32], F32)
            nc.sync.dma_start(wsb[:], wpack[:])
            wrb = constp.tile([H, 4 * H], F32R)
            nc.sync.dma_start(wrb[:], wpackr[:])
            w_ma = wrb[:, 0 * H:1 * H]
            w_mb = wrb[:, 1 * H:2 * H]
            w_aa = wrb[:, 2 * H:3 * H]
            w_ab = wrb[:, 3 * H:4 * H]
            btauM = wsb[:, 4 * H + 0:4 * H + 1]
            btauA = wsb[:, 4 * H + 1:4 * H + 2]
            c0 = 4 * H + 2
            st = [wsb[:, c0 + b * 2 * FB: c0 + (b + 1) * 2 * FB] for b in range(NBLK)]
            str_ = []
            for b in range(NBLK):
                r0 = staterp.tile([H, 2 * FB], F32R, tag=f"str{b}",
                                  name=f"str{b}_init")
                nc.vector.tensor_scalar(r0[:], st[b], 0.0, None, ALU.add)
                str_.append(r0)

            bsb = constp.tile([H, NPBF], BF16)
            nc.sync.dma_start(bsb[:], bpack[:])
            w_xb = bsb[:, 0:H]
            spk = [bsb[:, H + b * FB:H + (b + 1) * FB] for b in range(NBLK)]
            wx2 = constp.tile([10, H], BF16)
            nc.sync.dma_start(wx2[:], wx2d[:])

            psM = [psp.tile([H, FB], F32, tag=f"psM{b}", name=f"psM{b}") for b in range(NBLK)]
            psA = [psp.tile([H, FB], F32, tag=f"psA{b}", name=f"psA{b}") for b in range(NBLK)]

            xbh = [None, None]
            pending_mirror = []
            for t in range(S):
                if t % x_chunk == 0:
                    n_steps = min(x_chunk, S - t)
                    for b in range(NBLK):
                        xbh[b] = xinp.tile([10, x_chunk * FB], BF16, tag=f"xh{b}", name=f"xh{b}_{t}")
                        nc.sync.dma_start(xbh[b][:, 0:n_steps * FB],
                                          xts[b][:, t * FB:(t + n_steps) * FB])
                for b in range(NBLK):
                    bs = st[b][:, 0:FB]
                    mem = st[b][:, FB:2 * FB]
                    bsr = str_[b][:, 0:FB]
                    memr = str_[b][:, FB:2 * FB]
                    spk_b = spk[b]
                    co = (t % x_chunk) * FB
                    xt = xbh[b][:, co:co + FB]

                    # psX = (W@x + b1x) via K=10 packed trick + Wxb@spk
                    psX = psxp.tile([H, FB], F32, tag=f"psX{b}", name=f"psX{b}_{t}")
                    nc.tensor.matmul(psX[:], wx2[:], xt, start=True, stop=False)
                    nc.tensor.matmul(psX[:], w_xb, spk_b[:], start=False, stop=True)

                    dense = densep.tile([H, FB], F32, tag=f"dense{b}")
                    nc.scalar.activation(dense[:], psX[:], AF.Copy)
                    denser = densep.tile([H, FB], F32R, tag=f"denser{b}",
                                         name=f"denser{b}_{t}")
                    nc.scalar.activation(denser[:], psX[:], AF.Copy)

                    nc.tensor.matmul(psM[b][:], w_mb, memr, start=True, stop=False)
                    nc.tensor.matmul(psA[b][:], w_ab, bsr, start=True, stop=False)
                    nc.tensor.matmul(psM[b][:], w_ma, denser[:], start=False, stop=True)
                    nc.tensor.matmul(psA[b][:], w_aa, denser[:], start=False, stop=True)

                    tauMA = taup.tile([H, 2 * FB], F32, tag=f"tau{b}")  # [tauA | tauM]
                    nc.scalar.activation(tauMA[:, FB:2 * FB], psM[b][:], AF.Sigmoid, bias=btauM)
                    nc.scalar.activation(tauMA[:, 0:FB], psA[b][:], AF.Sigmoid, bias=btauA)
                    # deferred f32r mirror of the PREVIOUS block's state: emitted
                    # here (after this block's sigmoids) so it does not act as a
                    # cross-block barrier in the in-order ACT queue
                    for _dst, _src in pending_mirror:
                        nc.scalar.activation(_dst, _src, AF.Copy)
                    pending_mirror.clear()

                    dd = ppp.tile([H, 2 * FB], F32, tag=f"dd{b}")  # [d1s | d2]
                    nc.vector.scalar_tensor_tensor(
                        dd[:, 0:FB], spk_b[:], -BETA, bs, ALU.mult, ALU.add)
                    nc.vector.scalar_tensor_tensor(
                        dd[:, FB:2 * FB], dense[:], -1.0, mem, ALU.mult, ALU.add)
                    pp = ppp.tile([H, 2 * FB], F32, tag=f"pp{b}")  # [p1s | p2]
                    nc.vector.scalar_tensor_tensor(pp[:], tauMA[:], 1.0, dd[:], ALU.mult, ALU.mult)
                    p1s = pp[:, 0:FB]
                    p2 = pp[:, FB:2 * FB]

                    st_n = statep.tile([H, 2 * FB], F32, tag=f"state{b}")
                    bs_n = st_n[:, 0:FB]
                    mem_n = st_n[:, FB:2 * FB]
                    s1 = tmpp.tile([H, FB], F32, tag=f"s1{b}")
                    nc.vector.scalar_tensor_tensor(s1[:], dense[:], 1.0, p2, ALU.mult, ALU.add)
                    # t6 = Bth*spk = (p1s + BETA + B_J0) * spk   (spk in {0,1})
                    t6 = tmpp.tile([H, FB], F32, tag=f"t6{b}")
                    nc.vector.scalar_tensor_tensor(
                        t6[:], p1s, BETA + B_J0, spk_b[:], ALU.add, ALU.mult)
                    nc.vector.scalar_tensor_tensor(
                        bs_n, spk_b[:], BETA, p1s, ALU.mult, ALU.add)
                    # flush tiny bb values away from denormal range (DVE slowdown);
                    # Bth = 0.01 + bs so a 1e-10 floor is numerically invisible
                    nc.vector.tensor_scalar(bs_n, bs_n, 1e-10, None, ALU.max)
                    nc.vector.scalar_tensor_tensor(
                        mem_n, t6[:], -1.0, s1[:], ALU.mult, ALU.add)
                    spk_n = spkp.tile([H, FB], BF16, tag=f"spk{b}")
                    nc.vector.scalar_tensor_tensor(
                        spk_n[:], bs_n, B_J0, mem_n, ALU.add, ALU.is_lt)

                    str_n = staterp.tile([H, 2 * FB], F32R, tag=f"str{b}",
                                         name=f"strn{b}_{t}")
                    if t < S - 1:
                        pending_mirror.append((str_n[:], st_n[:]))

                    spk[b] = spk_n
                    st[b] = st_n
                    str_[b] = str_n

            for b in range(NBLK):
                nc.sync.dma_start(mem_out[:, b * FB:(b + 1) * FB], st[b][:, FB:2 * FB])

    _split_multiwaits(nc)
    return nc


def _bf16_split(a):
    hi = a.astype(ml_dtypes.bfloat16)
    lo = (a - hi.astype(np.float32)).astype(ml_dtypes.bfloat16)
    return hi, lo


def _prep_inputs_per_core(inputs):
    x = np.asarray(inputs["x"], np.float32)          # [S, B, 2]
    W1x = np.asarray(inputs["W1x"], np.float32)
    b1x = np.asarray(inputs["b1x"], np.float32)
    WtauM = np.asarray(inputs["WtauM"], np.float32)
    WtauAdp = np.asarray(inputs["WtauAdp"], np.float32)
    biases = np.stack([np.asarray(inputs["btauM"], np.float32),
                       np.asarray(inputs["btauAdp"], np.float32)], axis=1)
    wpack_l = [WtauM[:, :128].T, WtauM[:, 128:].T,
               WtauAdp[:, :128].T, (WtauAdp[:, 128:] / BETA).T, biases]
    wxb = np.ascontiguousarray(W1x[:, 2:].T).astype(ml_dtypes.bfloat16)
    h0_mem = np.asarray(inputs["h0_mem"], np.float32)
    h0_spk = np.asarray(inputs["h0_spk"], np.float32)
    h0_b = np.asarray(inputs["h0_b"], np.float32)

    # K=10: [Wh;Wh;Wl;Wl;b1h;b1l] x [xh;xl;xh;xl;1;1] = W@x + b1x exactly
    w2h, w2l = _bf16_split(np.ascontiguousarray(W1x[:, :2].T, np.float32))
    b1h, b1l = _bf16_split(b1x)
    wx2 = np.ascontiguousarray(np.concatenate(
        [w2h, w2h, w2l, w2l, b1h[None, :], b1l[None, :]], axis=0))
    xT = x.transpose(2, 0, 1)                                # [2, S, B]
    xh, xl = _bf16_split(xT)
    onesrow = np.ones((2, S * FB), dtype=ml_dtypes.bfloat16)

    in_maps = []
    for c in range(N_CORES):
        m = {"wx2": wx2}
        p32 = list(wpack_l)
        pbf = [wxb]
        for b in range(NBLK):
            sl = slice(c * BSH + b * FB, c * BSH + (b + 1) * FB)
            xpk = np.concatenate([xh[:, :, sl], xl[:, :, sl],
                                  xh[:, :, sl], xl[:, :, sl]], axis=0)
            m[f"xt{b}"] = np.ascontiguousarray(
                np.concatenate([xpk.reshape(8, S * FB), onesrow], axis=0))
            p32.append(np.concatenate([(BETA * h0_b[sl]).T, h0_mem[sl].T], axis=1))
            pbf.append(h0_spk[sl].T.astype(ml_dtypes.bfloat16))
        m["wpack"] = np.ascontiguousarray(np.concatenate(p32, axis=1).astype(np.float32))
        m["wpackr"] = np.ascontiguousarray(m["wpack"][:, :4 * H])
        m["bpack"] = np.ascontiguousarray(
            np.concatenate([np.asarray(a, dtype=ml_dtypes.bfloat16) for a in pbf], axis=1))
        in_maps.append(m)
    return in_maps


_NC_CACHE = {}


def _get_nc():
    if "nc" not in _NC_CACHE:
        _NC_CACHE["nc"] = _build_nc()
    return _NC_CACHE["nc"]


def _run(inputs, trace=False):
    from concourse.bass_utils import run_bass_kernel_spmd
    nc = _get_nc()
    in_maps = _prep_inputs_per_core(inputs)
    res = run_bass_kernel_spmd(nc, in_maps, core_ids=list(range(N_CORES)),
                               trace=trace)
    return res


def _finish_host(results, inputs):
    Wlin = np.asarray(inputs["Wlin"], np.float32)
    blin = np.asarray(inputs["blin"], np.float32)
    y = np.asarray(inputs["y"], np.float32)
    mem = np.concatenate([r["mem_out"].T for r in results], axis=0)  # [B, H]
    out = (mem @ Wlin.T + blin)[:, 0]
    return np.float32(np.mean((out.astype(np.float32) - y) ** 2, dtype=np.float32))


def kernel(x, y, h0_mem, h0_spk, h0_b, W1x, b1x, WtauM, btauM, WtauAdp,
           btauAdp, Wlin, blin):
    """Full (unsharded) inputs -> full scalar loss, computed on 8 TRN2 cores."""
    inputs = dict(x=x, y=y, h0_mem=h0_mem, h0_spk=h0_spk, h0_b=h0_b,
                  W1x=W1x, b1x=b1x, WtauM=WtauM, btauM=btauM,
                  WtauAdp=WtauAdp, btauAdp=btauAdp, Wlin=Wlin, blin=blin)
    res = _run(inputs, trace=False)
    return _finish_host(res.results, inputs)


def kernel_profiled(**inputs):
    """Like kernel(), but also returns neuron-profile exec time in ns."""
    res = _run(inputs, trace=True)
    return _finish_host(res.results, inputs), res.exec_time_ns

